# revision 9
# baseline (speedup 1.0000x reference)
"""DeepseekMoEGate routing kernel for 8 Trainium2 NeuronCores.

Sharding: data-parallel over the token dim (16384 -> 8 x 2048); the small
gate weight (256x2048) is replicated (transposed on host so the contraction
dim lands on SBUF partitions). Each core computes its shard's gate logits
(fp32r GEMM at full PE rate), grouped top-k routing (DVE max8/max_index),
top-k softmax weights, and a per-shard per-expert softmax-prob column sum.

fp32r is a tf32-like 20-bit format (11-bit mantissa), which perturbs logits
by ~1e-4. That can flip top-k selections only where two candidates are
within ~1e-3 of each other, so the device also emits each token's per-group
top-8 prob values; the host detects such near-tie tokens (~3% of tokens)
and re-routes just those exactly in float64. Cross-shard loss reductions
(2x256 floats/core) also run on host.
"""

import numpy as np

try:
    import concourse.bacc as bacc
except ImportError:  # fresh grading dir: make the repo importable
    import sys

    for p in ("/opt/trn_rl_repo", "/root/.axon_site/_ro/trn_rl_repo"):
        sys.path.insert(0, p)
    import concourse.bacc as bacc

import concourse.mybir as mybir
from concourse import bass_utils
from concourse.tile import TileContext

TOKENS = 16384
HIDDEN = 2048
E = 256            # experts
TOP_K = 8
N_GROUP = 8
TOPK_GROUP = 4
GS = E // N_GROUP  # 32 experts per group
AUX_ALPHA = 0.001
Z_ALPHA = 0.0001

N_CORES = 8
T_PER = TOKENS // N_CORES   # 2048 tokens per core
T_BLK = 512                 # tokens per DMA block
N_BLK = T_PER // T_BLK      # 4
SUB = T_BLK // 128          # 4 token sub-tiles per block
KC = HIDDEN // 128          # 16 contraction chunks
N_TILES = T_PER // 128      # 16 token tiles per core

# packed per-token output row: idx u32 x8 | weights f32 x8 | gmax f32 x64
OUT_COLS = TOP_K + TOP_K + N_GROUP * 8  # 80
GM_OFF = TOP_K + TOP_K                  # 16

# near-tie margin (logit domain) above which fp32r selection is provably
# right: observed pair-gap error sigma is ~1.6e-4, tau is ~6 sigma
TAU = 1.0e-3

F32 = mybir.dt.float32
F32R = mybir.dt.float32r
U32 = mybir.dt.uint32

_CACHE = {}


def _build_nc():
    nc = bacc.Bacc("TRN2", target_bir_lowering=False)
    # host pre-packs x and w so every DMA descriptor is a 16KB contiguous
    # run per partition (peak HBM rate):
    #   xP[p, b, k, t] = x_shard[b*T_BLK + t, k*128 + p]
    #   wP[p, k, e]    = gate_w[e, k*128 + p]
    xP = nc.dram_tensor("xP", [128, N_BLK * KC * T_BLK], F32R, kind="ExternalInput")
    wP = nc.dram_tensor("wP", [128, KC * E], F32R, kind="ExternalInput")
    o_all = nc.dram_tensor("o_all", [T_PER, OUT_COLS], F32, kind="ExternalOutput")
    o_ps = nc.dram_tensor("o_ps", [1, E], F32, kind="ExternalOutput")

    with TileContext(nc) as tc:
        with (
            tc.tile_pool(name="wpool", bufs=1) as wpool,
            tc.tile_pool(name="xpool", bufs=2) as xpool,
            tc.tile_pool(name="work", bufs=3) as work,
            tc.tile_pool(name="outp", bufs=4) as outp,
            tc.tile_pool(name="lgps", bufs=3, space="PSUM") as lgps_pool,
            tc.tile_pool(name="probps", bufs=1, space="PSUM") as prob_pool,
            tc.tile_pool(name="warmps", bufs=1, space="PSUM") as warm_pool,
        ):
            # PE warm-up: ~5us of dependency-free matmuls at kernel start so
            # the HAM clock gate reaches 2.4GHz while the first DMAs fill
            warm_sb = wpool.tile([128, 512], F32R)
            nc.vector.memset(warm_sb.bitcast(U32), 0)
            warm_ps = warm_pool.tile([128, 512], F32)
            for _ in range(12):
                nc.tensor.matmul(warm_ps, warm_sb[:, 0:128], warm_sb)

            # weight halves on the second HWDGE ring (scalar) so the first
            # matmuls only wait for the first 1MB
            w_sb = wpool.tile([128, KC, E], F32R)
            w_src = wP.rearrange("p (k e) -> p k e", k=KC)
            nc.scalar.dma_start(w_sb[:, 0 : KC // 2, :], w_src[:, 0 : KC // 2, :])
            nc.scalar.dma_start(w_sb[:, KC // 2 :, :], w_src[:, KC // 2 :, :])

            prob_ps = prob_pool.tile([1, E], F32)
            x_src = xP.rearrange("p (b k t) -> p b k t", b=N_BLK, k=KC)

            mm_i = 0
            for b in range(N_BLK):
                x_sb = xpool.tile([128, KC, T_BLK], F32R, tag="x_sb")
                for h in range(2):
                    nc.sync.dma_start(
                        x_sb[:, 8 * h : 8 * h + 8, :],
                        x_src[:, b, 8 * h : 8 * h + 8, :],
                    )
                for s in range(SUB):
                    t0 = b * T_BLK + s * 128
                    lg = lgps_pool.tile([128, E], F32, tag="lg")
                    for k in range(KC):
                        nc.tensor.matmul(
                            lg,
                            x_sb[:, k, s * 128 : (s + 1) * 128],
                            w_sb[:, k, :],
                            start=(k == 0),
                            stop=(k == KC - 1),
                        )

                    # P = exp(logits), S = per-token sum of P (softmax denom).
                    # No max-subtraction needed: |logit| <~ 7 here, exp is
                    # safely in fp32 range and softmax is shift-invariant.
                    P = work.tile([128, E], F32, tag="P")
                    S = work.tile([128, 1], F32, tag="S")
                    nc.scalar.activation(
                        P, lg, mybir.ActivationFunctionType.Exp, accum_out=S
                    )
                    R = work.tile([128, 1], F32, tag="R")
                    nc.vector.reciprocal(R, S)
                    # prob_ps[e] += sum_t P[t, e] / S[t]  (contract over the
                    # 128 tokens on partitions with a 1-column stationary op)
                    nc.tensor.matmul(
                        prob_ps,
                        R,
                        P,
                        start=(mm_i == 0),
                        stop=(mm_i == N_TILES - 1),
                        skip_group_check=True,
                    )
                    mm_i += 1

                    # packed output row: [idx u32 x8 | w f32 x8 | gmax f32 x64]
                    o_sb = outp.tile([128, OUT_COLS], F32, tag="o_sb")
                    gmax = o_sb[:, GM_OFF:]
                    # per-group top-8 (top-4 of each become the candidates)
                    for g in range(N_GROUP):
                        nc.vector.max(
                            gmax[:, 8 * g : 8 * (g + 1)], P[:, GS * g : GS * (g + 1)]
                        )
                    # gather each group's top-4 -> 32 candidate values
                    cand = work.tile([128, N_GROUP * TOPK_GROUP], F32, tag="cand")
                    nc.scalar.copy(
                        cand.rearrange("p (g c) -> p g c", c=TOPK_GROUP),
                        gmax.rearrange("p (g c) -> p g c", c=8)[:, :, 0:TOPK_GROUP],
                    )
                    top8 = work.tile([128, 8], F32, tag="top8")
                    nc.vector.max(top8, cand)
                    nc.vector.max_index(o_sb[:, 0:TOP_K].bitcast(U32), top8, P)
                    # topk weights: P_i / sum(top8 P)
                    s8 = work.tile([128, 1], F32, tag="s8")
                    nc.vector.reduce_sum(s8, top8, axis=mybir.AxisListType.X)
                    r8 = work.tile([128, 1], F32, tag="r8")
                    nc.vector.reciprocal(r8, s8)
                    nc.vector.tensor_scalar_mul(o_sb[:, TOP_K : 2 * TOP_K], top8, r8)

                    # packed output via the (idle) gpsimd SWDGE path
                    nc.gpsimd.dma_start(o_all[t0 : t0 + 128, :], o_sb)

            ps_sb = work.tile([1, E], F32, tag="ps_sb")
            nc.vector.tensor_copy(ps_sb, prob_ps)
            nc.gpsimd.dma_start(o_ps[:, :], ps_sb)

    nc.compile()
    return nc


def _get_nc():
    if "nc" not in _CACHE:
        _CACHE["nc"] = _build_nc()
    return _CACHE["nc"]


def _fp32r_round(a: np.ndarray) -> np.ndarray:
    """Round fp32 to fp32r (11-bit mantissa, low 12 bits zero), RNE."""
    b = np.ascontiguousarray(a, np.float32).view(np.uint32)
    low = b & np.uint32(0xFFF)
    hi = b & np.uint32(0xFFFFF000)
    round_up = (low > 0x800) | ((low == 0x800) & (((hi >> np.uint32(12)) & 1) == 1))
    hi = hi + (round_up.astype(np.uint32) << np.uint32(12))
    return hi.view(np.float32)


def _route_exact(logits64: np.ndarray):
    """Reference-equivalent grouped top-k routing, vectorized numpy.

    logits64: [n, 256] float64 (float32 logits widened). Returns idx [n, 8]
    int32 and weights [n, 8] float32, matching jax.lax.top_k tie-breaking
    (stable: lower index first on equal values).
    """
    n = logits64.shape[0]
    gl = logits64.reshape(n, N_GROUP, GS)
    # top-4 per group, stable on ties -> sort on (-value, index)
    order = np.argsort(-gl, axis=2, kind="stable")[:, :, :TOPK_GROUP]
    g_vals = np.take_along_axis(gl, order, axis=2)
    cand_vals = g_vals.reshape(n, N_GROUP * TOPK_GROUP)
    cand_idx = order.reshape(n, N_GROUP * TOPK_GROUP)
    pos = np.argsort(-cand_vals, axis=1, kind="stable")[:, :TOP_K]
    topk_vals = np.take_along_axis(cand_vals, pos, axis=1)
    sel_group = pos // TOPK_GROUP
    expert_in_group = np.take_along_axis(cand_idx, pos, axis=1)
    idx = (sel_group * GS + expert_in_group).astype(np.int32)
    # softmax over the selected logits (fp32-cast logits like the reference)
    tv32 = topk_vals.astype(np.float32).astype(np.float64)
    ex = np.exp(tv32 - tv32[:, :1])
    wgt = (ex / ex.sum(axis=1, keepdims=True)).astype(np.float32)
    return idx, wgt


def _uncertain_tokens(gm: np.ndarray) -> np.ndarray:
    """Tokens whose fp32r top-k selection might differ from fp32.

    gm: [n, 8, 8] per-group top-8 prob values (descending). A token is
    uncertain when any ordering decision among its top-9 candidates, or any
    group's candidate-cutoff (4th vs 5th) that could reach the top-8, is
    closer than TAU in logit space (ratio > exp(-TAU) in prob space).
    """
    thr = np.float32(np.exp(-TAU))
    cand = gm[:, :, :TOPK_GROUP].reshape(-1, N_GROUP * TOPK_GROUP)
    cs = np.sort(cand, axis=1)[:, ::-1]  # descending, [n, 32]
    top9 = cs[:, : TOP_K + 1]
    # adjacent-order near-ties among top-9 (affects selection and ordering)
    adj = top9[:, 1:] >= top9[:, :-1] * thr
    # group 4th/5th near-tie where the 5th could displace into the top-8
    g4 = gm[:, :, TOPK_GROUP - 1]
    g5 = gm[:, :, TOPK_GROUP]
    cutoff = cs[:, TOP_K - 1 : TOP_K] * thr
    grp = (g5 >= g4 * thr) & (g5 >= cutoff)
    return adj.any(axis=1) | grp.any(axis=1)


def kernel(hidden_states: np.ndarray, gate_w: np.ndarray):
    hidden_states = np.ascontiguousarray(hidden_states, dtype=np.float32)
    gate_w = np.ascontiguousarray(gate_w, dtype=np.float32)
    assert hidden_states.shape == (TOKENS, HIDDEN)
    assert gate_w.shape == (E, HIDDEN)

    nc = _get_nc()
    # wP[p, k, e] = gate_w[e, k*128+p]
    wP = _fp32r_round(
        np.ascontiguousarray(
            gate_w.T.reshape(KC, 128, E).transpose(1, 0, 2)
        ).reshape(128, KC * E)
    )
    in_maps = []
    for c in range(N_CORES):
        # xP[p, b, k, t] = x_shard[b*T_BLK+t, k*128+p]
        shard = hidden_states[c * T_PER : (c + 1) * T_PER]
        xP = _fp32r_round(
            np.ascontiguousarray(
                shard.reshape(N_BLK, T_BLK, KC, 128).transpose(3, 0, 2, 1)
            ).reshape(128, N_BLK * KC * T_BLK)
        )
        in_maps.append({"xP": xP, "wP": wP})

    res = bass_utils.run_bass_kernel_spmd(nc, in_maps, core_ids=list(range(N_CORES)))

    o_all = np.concatenate(
        [res.results[c]["o_all"] for c in range(N_CORES)], axis=0
    )
    topk_idx = o_all[:, 0:TOP_K].view(np.uint32).astype(np.int32)
    topk_weight = np.ascontiguousarray(o_all[:, TOP_K : 2 * TOP_K])
    gm = o_all[:, GM_OFF:].reshape(TOKENS, N_GROUP, 8)
    prob_sums = np.zeros(E, np.float32)
    for c in range(N_CORES):
        prob_sums += res.results[c]["o_ps"][0]

    # re-route near-tie tokens exactly (fp64 logits -> fp32, like reference)
    unc = np.flatnonzero(_uncertain_tokens(gm))
    if unc.size:
        logits = (
            hidden_states[unc].astype(np.float64) @ gate_w.T.astype(np.float64)
        ).astype(np.float32).astype(np.float64)
        fix_idx, fix_w = _route_exact(logits)
        topk_idx[unc] = fix_idx
        topk_weight[unc] = fix_w

    counts = np.bincount(topk_idx.reshape(-1), minlength=E).astype(np.float32)
    aux_loss = np.float32(np.sum(counts / TOKENS * (prob_sums / TOKENS)) * AUX_ALPHA)
    z_loss = np.float32(np.mean(np.log(prob_sums) ** 2) * Z_ALPHA)
    total_aux_loss = np.float32(aux_loss + z_loss)
    return topk_idx, topk_weight, total_aux_loss


# revision 13
# speedup vs baseline: 1.1264x; 1.1264x over previous
"""DeepseekMoEGate routing kernel for 8 Trainium2 NeuronCores.

Sharding: data-parallel over the token dim (16384 -> 8 x 2048); the small
gate weight (256x2048) is replicated (transposed on host so the contraction
dim lands on SBUF partitions). Each core computes its shard's gate logits
(fp32r GEMM at full PE rate), grouped top-k routing (DVE max8/max_index),
top-k softmax weights, and a per-shard per-expert softmax-prob column sum.

fp32r is a tf32-like 20-bit format (11-bit mantissa), which perturbs logits
by ~1e-4. That can flip top-k selections only where two candidates are
within ~1e-3 of each other, so the device also emits each token's per-group
top-8 prob values; the host detects such near-tie tokens (~3% of tokens)
and re-routes just those exactly in float64. Cross-shard loss reductions
(2x256 floats/core) also run on host.
"""

import numpy as np

try:
    import concourse.bacc as bacc
except ImportError:  # fresh grading dir: make the repo importable
    import sys

    for p in ("/opt/trn_rl_repo", "/root/.axon_site/_ro/trn_rl_repo"):
        sys.path.insert(0, p)
    import concourse.bacc as bacc

import concourse.mybir as mybir
from concourse import bass_utils
from concourse.tile import TileContext

TOKENS = 16384
HIDDEN = 2048
E = 256            # experts
TOP_K = 8
N_GROUP = 8
TOPK_GROUP = 4
GS = E // N_GROUP  # 32 experts per group
AUX_ALPHA = 0.001
Z_ALPHA = 0.0001

N_CORES = 8
T_PER = TOKENS // N_CORES   # 2048 tokens per core
T_BLK = 512                 # tokens per DMA block
N_BLK = T_PER // T_BLK      # 4
SUB = T_BLK // 128          # 4 token sub-tiles per block
KC = HIDDEN // 128          # 16 contraction chunks
N_TILES = T_PER // 128      # 16 token tiles per core

# packed per-token output row: idx u32 x8 | weights f32 x8 | gmax f32 x64
OUT_COLS = TOP_K + TOP_K + N_GROUP * 8  # 80
GM_OFF = TOP_K + TOP_K                  # 16

# near-tie margin (logit domain) above which fp32r selection is provably
# right: observed pair-gap error sigma is ~1.6e-4, tau is ~6 sigma
TAU = 1.0e-3

F32 = mybir.dt.float32
F32R = mybir.dt.float32r
U32 = mybir.dt.uint32

_CACHE = {}


def _build_nc():
    nc = bacc.Bacc("TRN2", target_bir_lowering=False)
    # host pre-packs x and w so every DMA descriptor is a 16KB contiguous
    # run per partition (peak HBM rate):
    #   xP[p, b, k, t] = x_shard[b*T_BLK + t, k*128 + p]
    #   wP[p, k, e]    = gate_w[e, k*128 + p]
    xP = nc.dram_tensor("xP", [128, N_BLK * KC * T_BLK], F32R, kind="ExternalInput")
    wP = nc.dram_tensor("wP", [128, KC * E], F32R, kind="ExternalInput")
    o_all = nc.dram_tensor("o_all", [T_PER, OUT_COLS], F32, kind="ExternalOutput")
    o_ps = nc.dram_tensor("o_ps", [1, E], F32, kind="ExternalOutput")

    with TileContext(nc) as tc:
        with (
            tc.tile_pool(name="wpool", bufs=1) as wpool,
            tc.tile_pool(name="xpool", bufs=3) as xpool,
            tc.tile_pool(name="work", bufs=3) as work,
            tc.tile_pool(name="outp", bufs=4) as outp,
            tc.tile_pool(name="lgps", bufs=3, space="PSUM") as lgps_pool,
            tc.tile_pool(name="probps", bufs=1, space="PSUM") as prob_pool,
            tc.tile_pool(name="warmps", bufs=1, space="PSUM") as warm_pool,
        ):
            # PE warm-up: dependency-free matmuls spanning the DMA-fill
            # window (~14us) so the HAM clock gate reaches 2.4GHz and stays
            # there when the first real matmuls arrive
            warm_sb = wpool.tile([128, 512], F32R)
            nc.vector.memset(warm_sb.bitcast(U32), 0)
            warm_ps = warm_pool.tile([128, 512], F32)
            for _ in range(48):
                nc.tensor.matmul(warm_ps, warm_sb[:, 0:128], warm_sb)

            # weight halves on the second HWDGE ring (scalar) so the first
            # matmuls only wait for the first 1MB
            w_sb = wpool.tile([128, KC, E], F32R)
            w_src = wP.rearrange("p (k e) -> p k e", k=KC)
            nc.scalar.dma_start(w_sb[:, 0 : KC // 2, :], w_src[:, 0 : KC // 2, :])
            nc.scalar.dma_start(w_sb[:, KC // 2 :, :], w_src[:, KC // 2 :, :])

            prob_ps = prob_pool.tile([1, E], F32)
            x_src = xP.rearrange("p (b k t) -> p b k t", b=N_BLK, k=KC)

            mm_i = 0
            for b in range(N_BLK):
                x_sb = xpool.tile([128, KC, T_BLK], F32R, tag="x_sb")
                for h in range(2):
                    nc.sync.dma_start(
                        x_sb[:, 8 * h : 8 * h + 8, :],
                        x_src[:, b, 8 * h : 8 * h + 8, :],
                    )
                for s in range(SUB):
                    t0 = b * T_BLK + s * 128
                    lg = lgps_pool.tile([128, E], F32, tag="lg")
                    for k in range(KC):
                        nc.tensor.matmul(
                            lg,
                            x_sb[:, k, s * 128 : (s + 1) * 128],
                            w_sb[:, k, :],
                            start=(k == 0),
                            stop=(k == KC - 1),
                        )

                    # P = exp(logits), S = per-token sum of P (softmax denom).
                    # No max-subtraction needed: |logit| <~ 7 here, exp is
                    # safely in fp32 range and softmax is shift-invariant.
                    P = work.tile([128, E], F32, tag="P")
                    S = work.tile([128, 1], F32, tag="S")
                    nc.scalar.activation(
                        P, lg, mybir.ActivationFunctionType.Exp, accum_out=S
                    )
                    R = work.tile([128, 1], F32, tag="R")
                    nc.vector.reciprocal(R, S)
                    # fp32r copies (ACT rounds on write) so the prob-sum
                    # matmul below runs at full PE rate instead of fp32's 4x
                    P2 = work.tile([128, E], F32R, tag="P2")
                    nc.scalar.activation(P2, lg, mybir.ActivationFunctionType.Exp)
                    R2 = work.tile([128, 1], F32R, tag="R2")
                    nc.scalar.copy(R2, R)
                    # prob_ps[e] += sum_t P[t, e] / S[t]  (contract over the
                    # 128 tokens on partitions with a 1-column stationary op)
                    nc.tensor.matmul(
                        prob_ps,
                        R2,
                        P2,
                        start=(mm_i == 0),
                        stop=(mm_i == N_TILES - 1),
                        skip_group_check=True,
                    )
                    mm_i += 1

                    # packed output row: [idx u32 x8 | w f32 x8 | gmax f32 x64]
                    o_sb = outp.tile([128, OUT_COLS], F32, tag="o_sb")
                    gmax = o_sb[:, GM_OFF:]
                    # per-group top-8 (top-4 of each become the candidates)
                    for g in range(N_GROUP):
                        nc.vector.max(
                            gmax[:, 8 * g : 8 * (g + 1)], P[:, GS * g : GS * (g + 1)]
                        )
                    # gather each group's top-4 -> 32 candidate values
                    cand = work.tile([128, N_GROUP * TOPK_GROUP], F32, tag="cand")
                    nc.vector.tensor_copy(
                        cand.rearrange("p (g c) -> p g c", c=TOPK_GROUP),
                        gmax.rearrange("p (g c) -> p g c", c=8)[:, :, 0:TOPK_GROUP],
                    )
                    top8 = work.tile([128, 8], F32, tag="top8")
                    nc.vector.max(top8, cand)
                    nc.vector.max_index(o_sb[:, 0:TOP_K].bitcast(U32), top8, P)
                    # topk weights: P_i / sum(top8 P)
                    s8 = work.tile([128, 1], F32, tag="s8")
                    nc.vector.reduce_sum(s8, top8, axis=mybir.AxisListType.X)
                    r8 = work.tile([128, 1], F32, tag="r8")
                    nc.vector.reciprocal(r8, s8)
                    nc.vector.tensor_scalar_mul(o_sb[:, TOP_K : 2 * TOP_K], top8, r8)

                    # packed output via the (idle) gpsimd SWDGE path
                    nc.gpsimd.dma_start(o_all[t0 : t0 + 128, :], o_sb)

            ps_sb = work.tile([1, E], F32, tag="ps_sb")
            nc.vector.tensor_copy(ps_sb, prob_ps)
            nc.gpsimd.dma_start(o_ps[:, :], ps_sb)

    nc.compile()
    return nc


def _get_nc():
    if "nc" not in _CACHE:
        _CACHE["nc"] = _build_nc()
    return _CACHE["nc"]


def _fp32r_round(a: np.ndarray) -> np.ndarray:
    """Round fp32 to fp32r (11-bit mantissa, low 12 bits zero), RNE."""
    b = np.ascontiguousarray(a, np.float32).view(np.uint32)
    low = b & np.uint32(0xFFF)
    hi = b & np.uint32(0xFFFFF000)
    round_up = (low > 0x800) | ((low == 0x800) & (((hi >> np.uint32(12)) & 1) == 1))
    hi = hi + (round_up.astype(np.uint32) << np.uint32(12))
    return hi.view(np.float32)


def _route_exact(logits64: np.ndarray):
    """Reference-equivalent grouped top-k routing, vectorized numpy.

    logits64: [n, 256] float64 (float32 logits widened). Returns idx [n, 8]
    int32 and weights [n, 8] float32, matching jax.lax.top_k tie-breaking
    (stable: lower index first on equal values).
    """
    n = logits64.shape[0]
    gl = logits64.reshape(n, N_GROUP, GS)
    # top-4 per group, stable on ties -> sort on (-value, index)
    order = np.argsort(-gl, axis=2, kind="stable")[:, :, :TOPK_GROUP]
    g_vals = np.take_along_axis(gl, order, axis=2)
    cand_vals = g_vals.reshape(n, N_GROUP * TOPK_GROUP)
    cand_idx = order.reshape(n, N_GROUP * TOPK_GROUP)
    pos = np.argsort(-cand_vals, axis=1, kind="stable")[:, :TOP_K]
    topk_vals = np.take_along_axis(cand_vals, pos, axis=1)
    sel_group = pos // TOPK_GROUP
    expert_in_group = np.take_along_axis(cand_idx, pos, axis=1)
    idx = (sel_group * GS + expert_in_group).astype(np.int32)
    # softmax over the selected logits (fp32-cast logits like the reference)
    tv32 = topk_vals.astype(np.float32).astype(np.float64)
    ex = np.exp(tv32 - tv32[:, :1])
    wgt = (ex / ex.sum(axis=1, keepdims=True)).astype(np.float32)
    return idx, wgt


def _uncertain_tokens(gm: np.ndarray) -> np.ndarray:
    """Tokens whose fp32r top-k selection might differ from fp32.

    gm: [n, 8, 8] per-group top-8 prob values (descending). A token is
    uncertain when any ordering decision among its top-9 candidates, or any
    group's candidate-cutoff (4th vs 5th) that could reach the top-8, is
    closer than TAU in logit space (ratio > exp(-TAU) in prob space).
    """
    thr = np.float32(np.exp(-TAU))
    cand = gm[:, :, :TOPK_GROUP].reshape(-1, N_GROUP * TOPK_GROUP)
    cs = np.sort(cand, axis=1)[:, ::-1]  # descending, [n, 32]
    top9 = cs[:, : TOP_K + 1]
    # adjacent-order near-ties among top-9 (affects selection and ordering)
    adj = top9[:, 1:] >= top9[:, :-1] * thr
    # group 4th/5th near-tie where the 5th could displace into the top-8
    g4 = gm[:, :, TOPK_GROUP - 1]
    g5 = gm[:, :, TOPK_GROUP]
    cutoff = cs[:, TOP_K - 1 : TOP_K] * thr
    grp = (g5 >= g4 * thr) & (g5 >= cutoff)
    return adj.any(axis=1) | grp.any(axis=1)


def kernel(hidden_states: np.ndarray, gate_w: np.ndarray):
    hidden_states = np.ascontiguousarray(hidden_states, dtype=np.float32)
    gate_w = np.ascontiguousarray(gate_w, dtype=np.float32)
    assert hidden_states.shape == (TOKENS, HIDDEN)
    assert gate_w.shape == (E, HIDDEN)

    nc = _get_nc()
    # wP[p, k, e] = gate_w[e, k*128+p]
    wP = _fp32r_round(
        np.ascontiguousarray(
            gate_w.T.reshape(KC, 128, E).transpose(1, 0, 2)
        ).reshape(128, KC * E)
    )
    in_maps = []
    for c in range(N_CORES):
        # xP[p, b, k, t] = x_shard[b*T_BLK+t, k*128+p]
        shard = hidden_states[c * T_PER : (c + 1) * T_PER]
        xP = _fp32r_round(
            np.ascontiguousarray(
                shard.reshape(N_BLK, T_BLK, KC, 128).transpose(3, 0, 2, 1)
            ).reshape(128, N_BLK * KC * T_BLK)
        )
        in_maps.append({"xP": xP, "wP": wP})

    res = bass_utils.run_bass_kernel_spmd(nc, in_maps, core_ids=list(range(N_CORES)))

    o_all = np.concatenate(
        [res.results[c]["o_all"] for c in range(N_CORES)], axis=0
    )
    topk_idx = o_all[:, 0:TOP_K].view(np.uint32).astype(np.int32)
    topk_weight = np.ascontiguousarray(o_all[:, TOP_K : 2 * TOP_K])
    gm = o_all[:, GM_OFF:].reshape(TOKENS, N_GROUP, 8)
    prob_sums = np.zeros(E, np.float32)
    for c in range(N_CORES):
        prob_sums += res.results[c]["o_ps"][0]

    # re-route near-tie tokens exactly (fp64 logits -> fp32, like reference)
    unc = np.flatnonzero(_uncertain_tokens(gm))
    if unc.size:
        logits = (
            hidden_states[unc].astype(np.float64) @ gate_w.T.astype(np.float64)
        ).astype(np.float32).astype(np.float64)
        fix_idx, fix_w = _route_exact(logits)
        topk_idx[unc] = fix_idx
        topk_weight[unc] = fix_w

    counts = np.bincount(topk_idx.reshape(-1), minlength=E).astype(np.float32)
    aux_loss = np.float32(np.sum(counts / TOKENS * (prob_sums / TOKENS)) * AUX_ALPHA)
    z_loss = np.float32(np.mean(np.log(prob_sums) ** 2) * Z_ALPHA)
    total_aux_loss = np.float32(aux_loss + z_loss)
    return topk_idx, topk_weight, total_aux_loss
